# revision 46
# baseline (speedup 1.0000x reference)
"""Fused MaskedCouplingRQS kernel for Trainium2, 8-core data parallel.

Self-contained: hardcodes all shapes. kernel(**inputs) takes the full inputs
(x [262144,64] plus MLP weights), shards the batch across 8 NeuronCores, runs
a fully fused Bass/Tile kernel (MLP + rational-quadratic-spline transform),
and returns (y [262144,64], logdet [262144]) matching the jax reference.

Per-core layout (batch on partitions for the spline phase):
  - chunks of 512 batch rows; MLP runs features-on-partitions (PE transposes
    x_masked tiles), final layer is flipped so raw spline params land
    [128 batch, 800] in PSUM.
  - spline phase: softmax/cumsum via ACT exp + tensor_tensor_scan (segmented
    cumsum with a carry mask), bin search via compares, gathers via
    onehot-multiply + free-axis reduce, then the RQS formula on [128,128] tiles.
"""
import numpy as np
from contextlib import ExitStack

import concourse.bass as bass
import concourse.bacc as bacc
import concourse.tile as tile
from concourse import mybir
from concourse.bass_utils import run_bass_kernel_spmd

F32 = mybir.dt.float32
AF = mybir.ActivationFunctionType
ALU = mybir.AluOpType
AX = mybir.AxisListType

N_CORES = 8
B_FULL = 262144
B_SHARD = B_FULL // N_CORES     # 32768
T = 32                          # transform dims
K = 8                           # spline bins
SP = 25                         # params per transform dim
HID = 128
RMIN, RMAX = -5.0, 5.0
MB = 1e-4                       # MIN_BIN
MS = 1e-4                       # MIN_SLOPE
CSCALE = (RMAX - RMIN) - K * MB
CHUNK = 512
NSUB = CHUNK // 128             # 4

USE_POOL = True                 # route some elementwise ops to GPSIMD
TRACE = False                   # set by test.py to get exec_time
LAST_RESULTS = None             # BassKernelResults stash for test.py

# All ACT functions used here (Relu/Exp/Ln/Square/Copy/Identity) live in the
# "natural_log_exp_and_others" table set, but bacc's greedy table-load pass
# picks exp_and_others for Exp and natural_log for Ln, reloading tables twice
# per chunk (~2.7us each). Blank out the earlier sets so the pass lands on the
# one covering set; ids/order preserved so walrus still resolves names right.
_orig_gat = bacc.get_activation_tables


def _gat_one_set(arch):
    t = _orig_gat(arch)
    out = {}
    for n, s in t.items():
        out[n] = set() if n in ("exp_and_others", "natural_log") else s
    return out


bacc.get_activation_tables = _gat_one_set


def build_program(n_chunks, repeats=1):
    nc = bacc.Bacc("TRN2", target_bir_lowering=False, debug=False,
                   num_devices=N_CORES)
    def eng2():
        return nc.gpsimd if USE_POOL else nc.vector

    b_shard = n_chunks * CHUNK
    x_d = nc.dram_tensor("x", [b_shard, 64], F32, kind="ExternalInput").ap()
    w1t_d = nc.dram_tensor("w1t", [32, HID], F32, kind="ExternalInput").ap()
    w2t_d = nc.dram_tensor("w2t", [HID, HID], F32, kind="ExternalInput").ap()
    w3t_d = nc.dram_tensor("w3t", [HID, HID], F32, kind="ExternalInput").ap()
    w4t_d = nc.dram_tensor("w4t", [HID, 800], F32, kind="ExternalInput").ap()
    b1_d = nc.dram_tensor("b1", [HID, 1], F32, kind="ExternalInput").ap()
    b2_d = nc.dram_tensor("b2", [HID, 1], F32, kind="ExternalInput").ap()
    b3_d = nc.dram_tensor("b3", [HID, 1], F32, kind="ExternalInput").ap()
    b4_d = nc.dram_tensor("b4hl", [2, 800], mybir.dt.bfloat16, kind="ExternalInput").ap()
    id_d = nc.dram_tensor("ident", [128, 128], F32, kind="ExternalInput").ap()
    carry_d = nc.dram_tensor("carry", [1, NSUB * T * K], F32, kind="ExternalInput").ap()
    kc_d = nc.dram_tensor("kc", [1, K - 1], F32, kind="ExternalInput").ap()
    y_d = nc.dram_tensor("y", [b_shard, 64], F32, kind="ExternalOutput").ap()
    ld_d = nc.dram_tensor("logdet", [b_shard], F32, kind="ExternalOutput").ap()

    with tile.TileContext(nc) as tc, ExitStack() as ctx:
        singles = ctx.enter_context(tc.tile_pool(name="singles", bufs=1))
        xpool = ctx.enter_context(tc.tile_pool(name="xpool", bufs=3))
        mlp = ctx.enter_context(tc.tile_pool(name="mlp", bufs=3))
        spl = ctx.enter_context(tc.tile_pool(name="spl", bufs=2))
        post = ctx.enter_context(tc.tile_pool(name="post", bufs=2))
        ps_t = ctx.enter_context(tc.tile_pool(name="ps_t", bufs=1, space="PSUM"))
        ps_h = ctx.enter_context(tc.tile_pool(name="ps_h", bufs=2, space="PSUM"))
        ps_p = ctx.enter_context(tc.tile_pool(name="ps_p", bufs=2, space="PSUM"))

        # ---- constants (loaded once) ----
        w1t = singles.tile([32, HID], F32)
        w2t = singles.tile([HID, HID], F32)
        w3t = singles.tile([HID, HID], F32)
        w4t = singles.tile([HID, 800], F32)
        b1 = singles.tile([HID, 1], F32)
        b2 = singles.tile([HID, 1], F32)
        b3 = singles.tile([HID, 1], F32)
        ident = singles.tile([128, 128], F32)
        carry = singles.tile([128, NSUB * T * K], F32)
        kc = singles.tile([128, K - 1], F32)
        b4hl = singles.tile([2, 800], mybir.dt.bfloat16)
        ones2 = singles.tile([2, 128], mybir.dt.bfloat16)
        nc.vector.memset(ones2, 1.0)
        for sb, dr in ((w1t, w1t_d), (w2t, w2t_d), (w3t, w3t_d), (w4t, w4t_d),
                       (b1, b1_d), (b2, b2_d), (b3, b3_d), (ident, id_d),
                       (b4hl, b4_d)):
            nc.sync.dma_start(out=sb, in_=dr)
        # broadcast [1, n] DRAM rows across 128 partitions
        for sb, dr in ((carry, carry_d), (kc, kc_d)):
            bc = bass.AP(tensor=dr.tensor, offset=dr.offset,
                         ap=[[0, 128]] + [list(d) for d in dr.ap[1:]])
            nc.sync.dma_start(out=sb, in_=bc)

        ld_acc = singles.tile([128, n_chunks, NSUB], F32)

        # const APs for ACT-side scalar adds (activation bias needs an AP)
        for vv in (MB, -RMIN, RMIN):
            key = (F32, vv)
            if key not in nc.const_aps.aps:
                ct = nc.alloc_sbuf_tensor(f"constap{len(nc.const_aps.aps)}",
                                          [128, 1], F32)
                nc.gpsimd.memset(ct.ap(), vv)
                nc.const_aps.aps[key] = ct.ap()

        for chunk in list(range(n_chunks)) * repeats:
            c0 = chunk * CHUNK
            xt = xpool.tile([128, NSUB, 64], F32)
            x_view = x_d[c0:c0 + CHUNK, :].rearrange("(c p) f -> p c f", p=128)
            nc.sync.dma_start(out=xt, in_=x_view)

            # ---- MLP (features on partitions) ----
            xT_ps = ps_t.tile([32, CHUNK], F32)
            for c in range(NSUB):
                nc.tensor.transpose(xT_ps[:, c * 128:(c + 1) * 128],
                                    xt[:, c, 0:32], ident)
            xT = mlp.tile([32, CHUNK], F32)
            nc.scalar.copy(xT, xT_ps)

            hp1 = ps_h.tile([128, CHUNK], F32, tag="hps")
            nc.tensor.matmul(hp1, w1t, xT, start=True, stop=True)
            h1 = mlp.tile([128, CHUNK], F32, tag="h1")
            nc.scalar.activation(h1, hp1, AF.Relu, bias=b1, scale=1.0)

            hp2 = ps_h.tile([128, CHUNK], F32, tag="hps")
            nc.tensor.matmul(hp2, w2t, h1, start=True, stop=True)
            h2 = mlp.tile([128, CHUNK], F32, tag="h2")
            nc.scalar.activation(h2, hp2, AF.Relu, bias=b2, scale=1.0)

            hp3 = ps_h.tile([128, CHUNK], F32, tag="hps")
            nc.tensor.matmul(hp3, w3t, h2, start=True, stop=True)
            h3 = mlp.tile([128, CHUNK], F32, tag="h3")
            nc.scalar.activation(h3, hp3, AF.Relu, bias=b3, scale=1.0)

            # ---- final layer flipped: raw params [128 batch, 800] in PSUM;
            # b4 bias folded in via a K=2 bf16 hi/lo accumulate-matmul.
            NCT = NSUB * T      # 128
            NTK = NCT * K       # 1024
            ew = spl.tile([128, NTK], F32, tag="ew")
            eh = spl.tile([128, NTK], F32, tag="eh")
            s_cont = spl.tile([128, NCT * (K + 1)], F32, tag="s_cont")
            for c in range(NSUB):
                pp = ps_p.tile([128, 800], F32, tag="pps")
                lhs = h3[:, c * 128:(c + 1) * 128]
                nc.tensor.matmul(pp[:, 0:512], lhs, w4t[:, 0:512],
                                 start=True, stop=False)
                nc.tensor.matmul(pp[:, 0:512], ones2, b4hl[:, 0:512],
                                 start=False, stop=True)
                nc.tensor.matmul(pp[:, 512:800], lhs, w4t[:, 512:800],
                                 start=True, stop=False)
                nc.tensor.matmul(pp[:, 512:800], ones2, b4hl[:, 512:800],
                                 start=False, stop=True)
                TK = T * K
                nc.scalar.activation(ew[:, c * TK:(c + 1) * TK],
                                     pp[:, 0:TK], AF.Exp)
                nc.scalar.activation(eh[:, c * TK:(c + 1) * TK],
                                     pp[:, TK:2 * TK], AF.Exp)
                nc.scalar.copy(s_cont[:, c * 288:(c + 1) * 288],
                               pp[:, 512:800])

            # ---- spline phase (chunk-wide, batch on partitions) ----
            # all tiles flat [128, NCT*...] with (c,t) merged so APs stay <=3D
            cwi = spl.tile([128, NTK], F32, tag="cwi")
            nc.vector.tensor_tensor_scan(cwi, carry, ew, 0.0, ALU.mult, ALU.add)


            def v3(t):  # [128, NTK] -> [128, NCT, K]
                return t.rearrange("p (g k) -> p g k", k=K)

            ew_v, eh_v, cwi_v = v3(ew), v3(eh), v3(cwi)
            s_v = s_cont.rearrange("p (g j) -> p g j", j=K + 1)

            xtr = xt[:, :, 32:64]             # [128, 4, 32] original x_trans
            xcc = post.tile([128, NCT], F32, tag="xcc")
            nc.vector.tensor_scalar(xcc.rearrange("p (c t) -> p c t", c=NSUB),
                                    xtr, RMIN, RMAX, ALU.max, ALU.min)

            recw = post.tile([128, NCT], F32, tag="recw")
            nc.vector.reciprocal(recw, cwi_v[:, :, 7])
            htot = post.tile([128, NCT], F32, tag="htot")
            nc.vector.tensor_reduce(htot, eh_v, AX.X, op=ALU.add)
            rech = post.tile([128, NCT], F32, tag="rech")
            nc.vector.reciprocal(rech, htot)

            # ge_j = xc >= x_pos_{j+1} = c*cwi_j*recw + (RMIN + MB*(j+1)), j=0..6
            u = spl.tile([128, NCT, K - 1], F32, tag="u")
            recw_b = recw.broadcast_to([128, NCT, K - 1])
            nc.vector.tensor_tensor(u, cwi_v[:, :, 0:7], recw_b, ALU.mult)
            v = spl.tile([128, NCT, K - 1], F32, tag="v")
            kc_b = bass.AP(tensor=kc.tensor, offset=kc.offset,
                           ap=[list(kc.ap[0]), [0, NCT], list(kc.ap[1])])
            nc.vector.scalar_tensor_tensor(v, u, CSCALE, kc_b, ALU.mult, ALU.add)
            ge = spl.tile([128, NCT, K - 1], F32, tag="ge")
            xcc_b = xcc.broadcast_to([128, NCT, K - 1])
            nc.vector.tensor_tensor(ge, xcc_b, v, ALU.is_ge)

            d = spl.tile([128, NCT, K], F32, tag="d")
            eng2().tensor_scalar(d[:, :, 0:1], ge[:, :, 0:1],
                                    -1.0, 1.0, ALU.mult, ALU.add)
            eng2().tensor_sub(d[:, :, 1:7], ge[:, :, 0:6], ge[:, :, 1:7])
            eng2().tensor_copy(d[:, :, 7:8], ge[:, :, 6:7])

            idx = post.tile([128, NCT], F32, tag="idx")
            nc.vector.tensor_reduce(idx, ge, AX.X, op=ALU.add)

            # gathers: sum_k V_k * d_k (onehot), and strict-cumsums directly
            # from ge: sum_{j<idx} e_j = sum_j e_j * ge_j (7-wide)
            gspec = (("gw", ew_v, d), ("gh", eh_v, d),
                     ("gs0", s_v[:, :, 0:K], d), ("gs1", s_v[:, :, 1:K + 1], d),
                     ("gcws", ew_v[:, :, 0:7], ge), ("gchs", eh_v[:, :, 0:7], ge))
            g = {}
            for gi, (name, src, sel) in enumerate(gspec):
                kk = src.shape[-1]
                p_t = spl.tile([128, NCT, K], F32, tag="pgat")
                me = nc.gpsimd if USE_POOL else nc.vector
                me.tensor_tensor(p_t[:, :, 0:kk], src, sel, ALU.mult)
                g[name] = post.tile([128, NCT], F32, tag=name, name=name)
                nc.vector.tensor_reduce(g[name], p_t[:, :, 0:kk], AX.X, op=ALU.add)

            def pt(tag):
                return post.tile([128, NCT], F32, tag=tag, name=tag)

            # ---- post: RQS formula on [128, 128] tiles ----
            gcws = g["gcws"]
            gchs = g["gchs"]
            swp = pt("swp"); nc.scalar.mul(swp, recw, CSCALE)
            shp = pt("shp"); nc.scalar.mul(shp, rech, CSCALE)

            wk_t = pt("wk_t"); eng2().tensor_mul(wk_t, g["gw"], swp)
            wk = pt("wk"); nc.scalar.add(wk, wk_t, MB)
            hk_t = pt("hk_t"); eng2().tensor_mul(hk_t, g["gh"], shp)
            hk = pt("hk"); nc.scalar.add(hk, hk_t, MB)

            a_x = pt("a_x"); eng2().tensor_mul(a_x, gcws, swp)
            b_x = pt("b_x")
            nc.vector.scalar_tensor_tensor(b_x, idx, MB, a_x, ALU.mult, ALU.add)
            xr = pt("xr"); nc.scalar.add(xr, xcc, -RMIN)
            numx = pt("numx"); nc.vector.tensor_sub(numx, xr, b_x)

            a_y = pt("a_y"); eng2().tensor_mul(a_y, gchs, shp)
            yk_t = pt("yk_t")
            nc.vector.scalar_tensor_tensor(yk_t, idx, MB, a_y, ALU.mult, ALU.add)
            yk = pt("yk"); nc.scalar.add(yk, yk_t, RMIN)

            rwk = pt("rwk"); nc.vector.reciprocal(rwk, wk)
            xi = pt("xi"); nc.vector.tensor_mul(xi, numx, rwk)
            om = pt("om")
            nc.scalar.activation(om, xi, AF.Identity, bias=1.0, scale=-1.0)
            t3 = pt("t3"); nc.vector.tensor_mul(t3, xi, om)
            sk = pt("sk"); nc.vector.tensor_mul(sk, hk, rwk)

            # slopes: dk = MS + ln(1 + exp(gs0))
            e0 = pt("e0"); nc.scalar.activation(e0, g["gs0"], AF.Exp)
            sp0 = pt("sp0"); nc.scalar.activation(sp0, e0, AF.Ln, bias=1.0)
            dk = pt("dk"); nc.scalar.add(dk, sp0, MS)
            e1 = pt("e1"); nc.scalar.activation(e1, g["gs1"], AF.Exp)
            sp1 = pt("sp1"); nc.scalar.activation(sp1, e1, AF.Ln, bias=1.0)
            dk1 = pt("dk1"); nc.scalar.add(dk1, sp1, MS)

            s2 = pt("s2"); nc.vector.tensor_add(s2, dk1, dk)
            s3 = pt("s3")
            nc.vector.scalar_tensor_tensor(s3, sk, -2.0, s2, ALU.mult, ALU.add)
            den_t = pt("den_t"); nc.vector.tensor_mul(den_t, s3, t3)
            den = pt("den"); nc.vector.tensor_add(den, den_t, sk)
            rden = pt("rden"); nc.vector.reciprocal(rden, den)

            xi2 = pt("xi2"); nc.scalar.activation(xi2, xi, AF.Square)
            m1 = pt("m1"); nc.vector.tensor_mul(m1, sk, xi2)
            m2 = pt("m2"); nc.vector.tensor_mul(m2, dk, t3)
            nume = pt("nume"); nc.vector.tensor_add(nume, m1, m2)
            u2 = pt("u2"); eng2().tensor_mul(u2, hk, rden)
            v2 = pt("v2"); nc.vector.tensor_mul(v2, u2, nume)
            y_in = pt("y_in"); nc.vector.tensor_add(y_in, v2, yk)

            # A = dk1*xi^2 + 2sk*xi*om + dk*om^2 = s3*xi^2 + 2(sk-dk)*xi + dk
            q1 = pt("q1"); nc.vector.tensor_mul(q1, s3, xi2)
            q2 = pt("q2"); eng2().tensor_sub(q2, sk, dk)
            q7 = pt("q7"); nc.vector.tensor_mul(q7, q2, xi)
            A0 = pt("A0")
            nc.vector.scalar_tensor_tensor(A0, q7, 2.0, q1, ALU.mult, ALU.add)
            A = pt("A"); nc.vector.tensor_add(A, A0, dk)
            z = pt("z"); nc.vector.tensor_mul(z, sk, rden)
            z2 = pt("z2"); nc.scalar.activation(z2, z, AF.Square)
            ldr = pt("ldr"); nc.vector.tensor_mul(ldr, A, z2)
            ld_in = pt("ld_in"); nc.scalar.activation(ld_in, ldr, AF.Ln)

            inside = pt("inside")
            nc.vector.tensor_tensor(inside.rearrange("p (c t) -> p c t", c=NSUB),
                                    xtr, xcc.rearrange("p (c t) -> p c t", c=NSUB),
                                    ALU.is_equal)
            inside_i = post.tile([128, NCT], mybir.dt.uint8, tag="inside_i")
            eng2().tensor_copy(inside_i, inside)
            nc.vector.copy_predicated(
                xt[:, :, 32:64],
                inside_i.rearrange("p (c t) -> p c t", c=NSUB),
                y_in.rearrange("p (c t) -> p c t", c=NSUB))
            ldm = pt("ldm"); nc.vector.tensor_mul(ldm, ld_in, inside)
            nc.vector.tensor_reduce(ld_acc[:, chunk, :],
                                    ldm.rearrange("p (c t) -> p c t", c=NSUB),
                                    AX.X, op=ALU.add)

            y_view = y_d[c0:c0 + CHUNK, :].rearrange("(c p) f -> p c f", p=128)
            nc.sync.dma_start(out=y_view, in_=xt)

        # ---- logdet writeback: transpose [128, G] -> [G, 128] then DMA ----
        GTOT = n_chunks * NSUB
        ld_flat = ld_acc.rearrange("p a b -> p (a b)")
        ld_view = ld_d.rearrange("(g p) -> g p", p=128)
        g0 = 0
        while g0 < GTOT:
            gn = min(128, GTOT - g0)
            tp = ps_t.tile([128, 128], F32, tag="warm", name="ldtp")
            nc.tensor.transpose(tp[0:gn, :], ld_flat[:, g0:g0 + gn], ident)
            sb = mlp.tile([128, 128], F32, tag="ldsb")
            nc.scalar.copy(sb[0:gn, :], tp[0:gn, :])
            nc.sync.dma_start(out=ld_view[g0:g0 + gn, :], in_=sb[0:gn, :])
            g0 += gn

    nc.compile()
    return nc


def _prep_consts(W1, b1, W2, b2, W3, b3, W4, b4):
    f = np.float32
    order = []
    for t in range(T):
        order += [t * SP + k for k in range(K)]
    for t in range(T):
        order += [t * SP + K + k for k in range(K)]
    for t in range(T):
        order += [t * SP + 2 * K + j for j in range(K + 1)]
    order = np.array(order)
    w4p = np.ascontiguousarray(W4[order, :].T, dtype=f)          # [128, 800]
    import ml_dtypes
    bf16 = ml_dtypes.bfloat16
    b4p = b4[order].astype(f)
    b4hi = b4p.astype(bf16)
    b4lo = (b4p - b4hi.astype(f)).astype(bf16)
    b4hl = np.ascontiguousarray(np.stack([b4hi, b4lo]))          # [2, 800] bf16
    carry = np.tile(np.array([0] + [1] * (K - 1), f), NSUB * T)[None, :]
    kcv = (RMIN + MB * np.arange(1, K, dtype=f))[None, :]
    return {
        "w1t": np.ascontiguousarray(W1.T, dtype=f),
        "w2t": np.ascontiguousarray(W2.T, dtype=f),
        "w3t": np.ascontiguousarray(W3.T, dtype=f),
        "w4t": w4p,
        "b1": np.ascontiguousarray(b1[:, None], dtype=f),
        "b2": np.ascontiguousarray(b2[:, None], dtype=f),
        "b3": np.ascontiguousarray(b3[:, None], dtype=f),
        "b4hl": b4hl,
        "ident": np.eye(128, dtype=f),
        "carry": np.ascontiguousarray(carry),
        "kc": np.ascontiguousarray(kcv),
    }


def kernel_sized(inputs, b_total):
    global LAST_RESULTS
    assert b_total % (N_CORES * CHUNK) == 0
    b_shard = b_total // N_CORES
    n_chunks = b_shard // CHUNK
    nc = build_program(n_chunks)
    consts = _prep_consts(inputs["W1"], inputs["b1"], inputs["W2"], inputs["b2"],
                          inputs["W3"], inputs["b3"], inputs["W4"], inputs["b4"])
    x = np.ascontiguousarray(inputs["x"][:b_total], dtype=np.float32)
    shards = x.reshape(N_CORES, b_shard, 64)
    in_maps = [dict(consts, x=np.ascontiguousarray(shards[i]))
               for i in range(N_CORES)]
    res = run_bass_kernel_spmd(nc, in_maps, core_ids=list(range(N_CORES)),
                               trace=TRACE)
    LAST_RESULTS = res
    y = np.concatenate([r["y"] for r in res.results], axis=0)
    ld = np.concatenate([r["logdet"] for r in res.results], axis=0)
    return y, ld


def kernel(**inputs):
    return kernel_sized(inputs, B_FULL)
